# revision 1
# baseline (speedup 1.0000x reference)
"""CrossSharedUnit Trainium2 kernel — 8-core data-parallel over batch.

Reference computation (per batch b, S=128 tokens, H=512 hidden, K=8):
  proj[b,s,k,g] = sum_h left[b,s,h] * G[h,k,g]
  raw[b,s,t,k]  = tanh(sum_g proj[b,s,k,g] * right[b,t,g])
  score[b,s,t]  = sum_k raw[b,s,t,k] * v[k]
  attn          = softmax(score, axis=t)
  out           = self + attn @ other_hidden
for two branches (aspect: left=aspect, right=polarity; polarity: left=aspect,
right=aspect — faithful to the source which uses aspect on both sides).

Sharding: batch B=32 split 4-per-core across 8 cores; G tensors replicated.
No collectives. Activations are shipped both natural [bs,h] and pre-transposed
[h,bs] from the host so every matmul contraction lands on the partition axis.

All matmuls run as float32r (TF32-like, 1 cycle/row at N>=512 vs 4 for fp32;
measured ~4e-4 rel err). Softmax needs no max-subtraction: |score| <= sum|v_k|
so exp() cannot overflow in fp32. The softmax division is deferred through the
attention matmul: out = self + (E @ other) / Z with Z from a ones-matmul,
applied as a per-partition scalar in the final fused DVE op.
"""

import os
import sys

sys.path.insert(0, "/opt/trn_rl_repo")

import numpy as np

from concourse import bacc, mybir, tile
from concourse.bass_utils import run_bass_kernel_spmd

B, S, H, K = 32, 128, 512, 8
NCORES = 8
BL = B // NCORES          # batches per core
BS = BL * S               # rows per core (512)
P = 128                   # partitions
HT = H // P               # h partition-tiles (4)
KG = K * H                # flattened (k,g) axis (4096)
F32 = mybir.dt.float32
F32R = mybir.dt.float32r

_cache = {}


def _build():
    """Build + compile the per-core Bass program (same program on all cores)."""
    nc = bacc.Bacc("TRN2", target_bir_lowering=False, debug=False,
                   num_devices=NCORES)

    xa_nat_d = nc.dram_tensor("xa_nat", [BS, H], F32R, kind="ExternalInput")
    xp_nat_d = nc.dram_tensor("xp_nat", [BS, H], F32R, kind="ExternalInput")
    xa_t_d = nc.dram_tensor("xa_t", [H, BS], F32R, kind="ExternalInput")
    xp_t_d = nc.dram_tensor("xp_t", [H, BS], F32R, kind="ExternalInput")
    g_ap_d = nc.dram_tensor("g_ap", [H, KG], F32R, kind="ExternalInput")
    g_pa_d = nc.dram_tensor("g_pa", [H, KG], F32R, kind="ExternalInput")
    v_ap_d = nc.dram_tensor("v_ap", [K, 1], F32, kind="ExternalInput")
    v_pa_d = nc.dram_tensor("v_pa", [K, 1], F32, kind="ExternalInput")
    out_a_d = nc.dram_tensor("out_a", [BS, H], F32, kind="ExternalOutput")
    out_p_d = nc.dram_tensor("out_p", [BS, H], F32, kind="ExternalOutput")

    with tile.TileContext(nc) as tc:
        with (
            tc.tile_pool(name="const", bufs=1) as cpool,
            tc.tile_pool(name="gk0p", bufs=2) as gk0p,
            tc.tile_pool(name="ghbp", bufs=1) as ghbp,
            tc.tile_pool(name="proj", bufs=1) as projpool,
            tc.tile_pool(name="work", bufs=3) as work,
            tc.tile_pool(name="ps_mm", bufs=3, space="PSUM") as ps_mm,
            tc.tile_pool(name="ps_z", bufs=1, space="PSUM") as ps_z,
            tc.tile_pool(name="ps_o", bufs=2, space="PSUM") as ps_o,
        ):
            # ---- persistent activations -------------------------------
            xa_nat = [cpool.tile([P, H], F32R, tag=f"xa_nat{i}", name=f"xa_nat{i}") for i in range(BL)]
            xp_nat = [cpool.tile([P, H], F32R, tag=f"xp_nat{i}", name=f"xp_nat{i}") for i in range(BL)]
            xa_t = [cpool.tile([P, BS], F32R, tag=f"xa_t{i}", name=f"xa_t{i}") for i in range(HT)]
            xp_t = [cpool.tile([P, BS], F32R, tag=f"xp_t{i}", name=f"xp_t{i}") for i in range(HT)]
            # Critical path to the first matmuls: xa_t + the first G tiles.
            # Spread issues across engine queues so they don't serialize
            # on one sequencer.
            engs = [nc.sync, nc.gpsimd, nc.scalar]
            fast = [nc.sync, nc.scalar]
            for i in range(HT):
                fast[i % 2].dma_start(out=xa_t[i][:], in_=xa_t_d.ap()[i * P:(i + 1) * P, :])

            # ---- v vectors broadcast across partitions ----------------
            vrow_a = cpool.tile([1, K], F32, tag="vrow_a")
            vrow_p = cpool.tile([1, K], F32, tag="vrow_p")
            nc.gpsimd.dma_start(out=vrow_a[:], in_=v_ap_d.ap().rearrange("k o -> o k"))
            nc.gpsimd.dma_start(out=vrow_p[:], in_=v_pa_d.ap().rearrange("k o -> o k"))
            vbc_a = cpool.tile([P, K], F32, tag="vbc_a")
            vbc_p = cpool.tile([P, K], F32, tag="vbc_p")
            nc.gpsimd.partition_broadcast(vbc_a[:], vrow_a[:])
            nc.gpsimd.partition_broadcast(vbc_p[:], vrow_p[:])

            ones_f = cpool.tile([P, 2], F32, tag="ones_f")
            nc.vector.memset(ones_f[:], 1.0)
            ones_t = cpool.tile([P, 2], F32R, tag="ones_t")
            nc.vector.tensor_copy(ones_t[:], ones_f[:])

            def load_rest():
                # non-critical loads (stage 2+) on gpsimd's own queue
                for i in range(HT):
                    nc.gpsimd.dma_start(out=xp_t[i][:], in_=xp_t_d.ap()[i * P:(i + 1) * P, :])
                for i in range(BL):
                    nc.gpsimd.dma_start(out=xa_nat[i][:], in_=xa_nat_d.ap()[i * P:(i + 1) * P, :])
                    nc.gpsimd.dma_start(out=xp_nat[i][:], in_=xp_nat_d.ap()[i * P:(i + 1) * P, :])

            branches = [
                # (G dram, right_t tiles, stage4 rhs (other), residual (self),
                #  vbc, out dram)
                (g_ap_d, xp_t, xp_nat, xa_nat, vbc_a, out_a_d),
                (g_pa_d, xa_t, xa_nat, xp_nat, vbc_p, out_p_d),
            ]

            for br, (g_d, right_t, nat_other, nat_self, vbc, out_d) in enumerate(branches):
                # ---- stage 1: projT2[g, (b,k,s)] = G.T @ leftT ----------
                # left is always the aspect tensor (faithful to source).
                projT2 = [projpool.tile([P, K, BL, S], F32R, tag=f"projT2_{gt}", name=f"projT2_{gt}")
                          for gt in range(HT)]
                # G loads: per-h tiles so each DMA moves 2KB/14KB contiguous
                # bursts per partition row. k=0 is a separate small tile so the
                # first matmul group doesn't wait on the 7MB bulk.
                PIECES = [(1, 3), (3, 5), (5, 7), (7, 8)]  # [k0, k1) ranges
                gk0 = [gk0p.tile([P, H], F32R, tag=f"gk0_{h}", name=f"gk0_{h}")
                       for h in range(HT)]
                ghb = [[ghbp.tile([P, (k1 - k0) * H], F32R,
                                  tag=f"ghb_{h}_{pi}", name=f"ghb_{h}_{pi}",
                                  bufs=2 if pi == 0 else 1)
                        for pi, (k0, k1) in enumerate(PIECES)]
                       for h in range(HT)]
                for h in range(HT):
                    fast[h % 2].dma_start(
                        out=gk0[h][:], in_=g_d.ap()[h * P:(h + 1) * P, 0:H])
                for pi, (k0, k1) in enumerate(PIECES):
                    for h in range(HT):
                        # last piece rides the otherwise-idle gpsimd queue
                        eng = nc.gpsimd if pi == 3 else fast[(h + pi) % 2]
                        eng.dma_start(
                            out=ghb[h][pi][:],
                            in_=g_d.ap()[h * P:(h + 1) * P, k0 * H:k1 * H])
                for k in range(K):
                    for gt in range(HT):
                        acc = ps_mm.tile([P, BL, S], F32, tag="mmacc", name="acc")
                        for h in range(HT):
                            if k == 0:
                                lhsT = gk0[h][:, gt * P:(gt + 1) * P]
                            else:
                                pi = (k - 1) // 2
                                off = (k - PIECES[pi][0]) * H + gt * P
                                lhsT = ghb[h][pi][:, off:off + P]
                            nc.tensor.matmul(
                                acc[:], lhsT, xa_t[h][:],
                                start=(h == 0), stop=(h == HT - 1))
                        # scalar's stream is busy issuing DMA descriptors for
                        # the first k's — route those evacuations to vector
                        if k < 2 or (k * HT + gt) % 2 == 1:
                            nc.vector.tensor_copy(projT2[gt][:, k, :, :], acc[:])
                        else:
                            nc.scalar.copy(projT2[gt][:, k, :, :], acc[:])

                if br == 0:
                    load_rest()

                # ---- stages 2-4 per batch -------------------------------
                NCK = 2            # k-chunks per batch
                KC = K // NCK      # k's per chunk (4)
                for b in range(BL):
                    th = []
                    for ck in range(NCK):
                        acc2 = ps_mm.tile([P, KC, S], F32, tag="mmacc", name="acc2")
                        for gi in range(HT):
                            nc.tensor.matmul(
                                acc2[:],
                                right_t[gi][:, b * S:(b + 1) * S],
                                projT2[gi][:, ck * KC:(ck + 1) * KC, b, :],
                                start=(gi == 0), stop=(gi == HT - 1))
                        t_sb = work.tile([P, KC, S], F32, tag="tanh", bufs=4)
                        nc.scalar.activation(t_sb[:], acc2[:],
                                             mybir.ActivationFunctionType.Tanh)
                        th.append(t_sb)
                    # weighted sum over k: scoreT[t,s] = sum_k v_k * tanh_k
                    # (two independent chains to shorten the serial path)
                    sca = work.tile([P, S], F32, tag="score_a")
                    scb = work.tile([P, S], F32, tag="score_b")
                    nc.vector.tensor_scalar_mul(sca[:], th[0][:, 0, :], vbc[:, 0:1])
                    nc.vector.tensor_scalar_mul(scb[:], th[1][:, 0, :], vbc[:, KC:KC + 1])
                    for j in range(1, KC):
                        nc.vector.scalar_tensor_tensor(
                            sca[:], th[0][:, j, :], vbc[:, j:j + 1],
                            sca[:], mybir.AluOpType.mult, mybir.AluOpType.add)
                        nc.vector.scalar_tensor_tensor(
                            scb[:], th[1][:, j, :], vbc[:, KC + j:KC + j + 1],
                            scb[:], mybir.AluOpType.mult, mybir.AluOpType.add)
                    sc = work.tile([P, S], F32, tag="score")
                    nc.vector.tensor_tensor(sc[:], sca[:], scb[:], mybir.AluOpType.add)
                    # E_T = exp(scoreT)   (|score| <= sum|v| so no overflow)
                    e_t = work.tile([P, S], F32R, tag="e_t")
                    nc.scalar.activation(e_t[:], sc[:],
                                         mybir.ActivationFunctionType.Exp)
                    # Z[s] = sum_t E_T[t,s]  via ones-matmul
                    zp = ps_z.tile([P, 2], F32, tag="z")
                    nc.tensor.matmul(zp[:], e_t[:], ones_t[:], start=True, stop=True)
                    rz = work.tile([P, 1], F32, tag="rz")
                    nc.vector.reciprocal(rz[:], zp[:, 0:1])
                    # out = self + (E_T.T @ other) / Z
                    rp = ps_o.tile([P, H], F32, tag="raw")
                    nc.tensor.matmul(rp[:], e_t[:], nat_other[b][:],
                                     start=True, stop=True)
                    ot = work.tile([P, H], F32, tag="out")
                    for half, eng in ((0, nc.sync), (1, nc.scalar)):
                        lo, hi = half * (H // 2), (half + 1) * (H // 2)
                        nc.vector.scalar_tensor_tensor(
                            ot[:, lo:hi], rp[:, lo:hi], rz[:, 0:1],
                            nat_self[b][:, lo:hi].bitcast(F32),
                            mybir.AluOpType.mult, mybir.AluOpType.add)
                        eng.dma_start(out=out_d.ap()[b * P:(b + 1) * P, lo:hi],
                                      in_=ot[:, lo:hi])

    nc.compile()
    return nc


def _get_nc():
    if "nc" not in _cache:
        _cache["nc"] = _build()
    return _cache["nc"]


def _prep_in_maps(aspect_hidden, polarity_hidden, G_aspect_polarity,
                  G_polarity_aspect, G_vector_aspect, G_vector_polarity):
    f = np.float32
    a = np.ascontiguousarray(aspect_hidden, dtype=f)
    p = np.ascontiguousarray(polarity_hidden, dtype=f)
    g_ap = np.ascontiguousarray(G_aspect_polarity, dtype=f).reshape(H, KG)
    g_pa = np.ascontiguousarray(G_polarity_aspect, dtype=f).reshape(H, KG)
    v_ap = np.ascontiguousarray(G_vector_aspect, dtype=f)
    v_pa = np.ascontiguousarray(G_vector_polarity, dtype=f)

    in_maps = []
    for c in range(NCORES):
        a_loc = a[c * BL:(c + 1) * BL].reshape(BS, H)
        p_loc = p[c * BL:(c + 1) * BL].reshape(BS, H)
        in_maps.append({
            "xa_nat": a_loc,
            "xp_nat": p_loc,
            "xa_t": np.ascontiguousarray(a_loc.T),
            "xp_t": np.ascontiguousarray(p_loc.T),
            "g_ap": g_ap,
            "g_pa": g_pa,
            "v_ap": v_ap,
            "v_pa": v_pa,
        })
    return in_maps


def kernel(aspect_hidden, polarity_hidden, G_aspect_polarity,
           G_polarity_aspect, G_vector_aspect, G_vector_polarity):
    nc = _get_nc()
    in_maps = _prep_in_maps(aspect_hidden, polarity_hidden, G_aspect_polarity,
                            G_polarity_aspect, G_vector_aspect,
                            G_vector_polarity)
    res = run_bass_kernel_spmd(
        nc, in_maps, core_ids=list(range(NCORES)),
        trace=bool(os.environ.get("KERNEL_TRACE")))
    _cache["last_results"] = res

    out_a = np.empty((B, S, H), np.float32)
    out_p = np.empty((B, S, H), np.float32)
    for c in range(NCORES):
        out_a[c * BL:(c + 1) * BL] = res.results[c]["out_a"].reshape(BL, S, H)
        out_p[c * BL:(c + 1) * BL] = res.results[c]["out_p"].reshape(BL, S, H)
    return (out_a, out_p)

